# revision 1
# baseline (speedup 1.0000x reference)
"""Trainium2 Bass kernel for nn_DGLossVer2 (gyro Huber loss + gaussian NLL).

Strategy (v2)
-------------
Data-parallel over batch N=128 across 8 NeuronCores (16 sequences/core).
Partition p holds a contiguous t-range of one sequence (128 = 16 seq x 8
chunks of T); all pairwise-tree work stays within a partition.

v2 rebalances the engines (v1 was vector-bound at ~70% of the span):
- NLL subtracts run on the (previously idle) GpSimd engine.
- The 4-instruction pairwise halving tree is ONE tensor_reduce over a
  strided [P, 3, g, 16] view (the 16-step axis innermost).
- Quaternion products are emitted as 6-8 fat strided multiplies into an
  interleaved product tile M[c, n, i] followed by a single reduce over
  the i axis, using an 8-row [g; -g] stack so all sign patterns become
  affine row-selections (index tables below).
- so3_log via quaternions in log space: theta/2 = arctan(exp(-.5|ln s2 -
  ln w2|)) (+ quadrant fixup), 1/|v| = exp(-.5 ln s2), so sqrt/divide
  never run on the slow DVE reciprocal and the whole log+NLL phase stays
  in the natural_log_exp activation table. 5 table loads total (v1: 8).
- The Huber sum is sign-free (|rs| only), batched over x/y/z as single
  [P, 3, n] ops with stride-0 broadcast of the scale plane.

Each core emits per-partition partial sums packed in one [128, 24] tile;
the host combines them (see combine()).
"""

import numpy as np

import concourse.bass as bass
import concourse.mybir as mybir
from concourse.bass import AP
from concourse.mybir import AluOpType as Op
from concourse.mybir import ActivationFunctionType as AF
from concourse.tile import TileContext

F32 = mybir.dt.float32
AX = mybir.AxisListType


def _patch_drain():
    """walrus codegen in this container rejects >1 sync wait on SP-engine
    instructions; spread the kernel-tail drain's waits across 1-wait NOPs."""
    from concourse import tile as tile_mod
    from concourse.vector_clock import ScopedClock

    if getattr(tile_mod.TileContext, "_drain_patched", False):
        return

    def _drain_and_barrier(self, tick_clock, wait_clock):
        nop0 = self.nc.sync.nop(nofuse=True)
        wait_clock.add_sem_waits(nop0.ins,
                                 ScopedClock({None: tick_clock.global_clock}))
        si = nop0.ins.sync_info
        if si is not None and len(si.on_wait) > 1:
            waits = list(si.on_wait)
            si.on_wait = waits[:1]
            for w in waits[1:]:
                nopn = self.nc.sync.nop(nofuse=True)
                nopn.ins.sync_info = mybir.SyncInfo(on_wait=[w], on_update=[])
        self.nc.sync.drain()
        self.nc.all_engine_barrier()
        assert self.sems is not None
        popped = self.nc._tile_sem_poison_stack.pop()
        assert popped is self._sem_poison
        self.nc.clear_and_free_semaphores(list(self.sems.allocated().values()))
        self.nc.all_engine_barrier()

    tile_mod.TileContext._drain_and_barrier = _drain_and_barrier
    tile_mod.TileContext._drain_patched = True


def _split_multi_waits(nc):
    """This container's walrus codegen allows only one sync wait per
    instruction; move extra waits onto same-engine NoOps inserted before."""
    n = 0
    for bb in nc.m.functions[0].blocks:
        new = []
        for inst in bb.instructions:
            si = inst.sync_info
            if si is not None and len(si.on_wait) > 1:
                waits = list(si.on_wait)
                for w in waits[:-1]:
                    n += 1
                    new.append(mybir.InstNoOp(
                        name=f"wsplit-{n}", engine=inst.engine,
                        sync_info=mybir.SyncInfo(on_wait=[w], on_update=[]),
                        bass_nofuse=True))
                si.on_wait = waits[-1:]
            new.append(inst)
        bb.instructions[:] = new
    return n


DT = 0.005
W_ = 1.0e6
H_ = 0.005
N0 = 5
EPS = 1e-6
PI = float(np.pi)

N_CORES = 8
N_FULL, T_FULL = 128, 16384
P = 128

# chunk schedule (t-steps per partition per chunk); 16-group ranges per chunk
SIZES = [256, 512, 768, 512]
# tail passes as 16-group ranges [g0, g1); must align with chunk boundaries
PASSES = [(0, 96), (96, 128)]
NCH = len(SIZES)

# quaternion-product row tables: out[c] = sum_i A[i] * S[sigma(c,i)] where
# S is the 8-row stack [q; -q].  Each entry: (i0, di, s0, ds, k) emits one
# multiply of k consecutive-i products with A-rows (i0 + j*di) and stack
# rows (s0 + j*ds).  All strides positive by construction.
# conj(h) (x) g   (residual)
ROWS_CONJ = {
    0: [(0, 1, 0, 1, 4)],
    1: [(0, 3, 1, 1, 2), (1, 1, 4, 3, 2)],
    2: [(0, 1, 2, 1, 4)],
    3: [(0, 1, 3, 3, 2), (2, 1, 1, 3, 2)],
}
# p (x) q         (g32 pairwise level)
ROWS_MUL = {
    0: [(0, 1, 0, 5, 2), (2, 1, 6, 1, 2)],
    1: [(0, 3, 1, 5, 2), (1, 1, 0, 3, 2)],
    2: [(0, 1, 2, 5, 2), (2, 1, 0, 1, 2)],
    3: [(0, 2, 3, 2, 2), (1, 1, 2, 1, 1), (3, 1, 0, 1, 1)],
}


def _flat(d):
    # [n_seq, T, 3] dram tensor -> [128, 3*L] AP (partition p = (seq, chunk-of-T))
    return d[:].flatten().rearrange("(p l) -> p l", p=P)


def build(n_seq=16, T=16384):
    sp = P // n_seq          # partitions per sequence
    L = T // sp              # t-steps per partition
    n16 = L // 16
    n32 = L // 32
    ncat = n16 + n32
    assert sum(SIZES) == L

    _patch_drain()
    nc = bass.Bass()
    for cname, cval in (("pi2", PI / 2), ("pi", PI), ("sqeps", 1e-3),
                    ("msqeps", -1e-3), ("kh", DT / 2)):
        _cc = nc.alloc_sbuf_tensor(f"const-f32-{cname}", [128, 1], F32)
        nc.gpsimd.memset(_cc.ap(), cval)
        nc.const_aps.aps[(F32, cval)] = _cc.ap()
    nc.all_engine_barrier()

    wh_d = nc.declare_dram_parameter("w_hat", [n_seq, T, 3], F32, isOutput=False)
    dw_d = nc.declare_dram_parameter("dw_16", [n_seq, T, 3], F32, isOutput=False)
    gt_d = nc.declare_dram_parameter("w_gt", [n_seq, T, 3], F32, isOutput=False)
    mn_d = nc.declare_dram_parameter("w_mean", [n_seq, T, 3], F32, isOutput=False)
    sd_d = nc.declare_dram_parameter("w_std", [n_seq, T, 3], F32, isOutput=False)
    mkc_d = nc.declare_dram_parameter("maskc", [P, ncat], F32, isOutput=False)
    out_d = nc.declare_dram_parameter("out", [P, 24], F32, isOutput=True)

    # pass-major column bases: pass p owns cols [base, base+w16p) (16-level)
    # and [base+w16p, base+w16p+w32p) (32-level)
    pbase, pw16, pw32 = [], [], []
    b = 0
    for (g0, g1) in PASSES:
        pbase.append(b)
        pw16.append(g1 - g0)
        pw32.append((g1 - g0) // 2)
        b += pw16[-1] + pw32[-1]
    assert b == ncat

    def col16(g):
        # global 16-group index -> pass-major column
        for p_, (g0, g1) in enumerate(PASSES):
            if g0 <= g < g1:
                return pbase[p_] + (g - g0)
        raise AssertionError(g)

    import os
    DBG_GP = os.environ.get("K_GP", "0") == "1"      # gpsimd tensor ops unsupported here
    DBG_TAIL = os.environ.get("K_TAIL", "1") == "1"  # emit gyro tail
    DBG_RED = os.environ.get("K_RED", "1") == "1"    # strided 16-group reduce
    DBG_BC = os.environ.get("K_BC", "1") == "1"      # stride-0 broadcast muls

    from contextlib import ExitStack
    with TileContext(nc) as tc, ExitStack() as _es:
        v = nc.vector
        act = nc.scalar
        gp = nc.gpsimd if DBG_GP else nc.vector
        pp = _es.enter_context(tc.tile_pool(name="persist", bufs=1))

        def ptile(shape, name):
            return pp.tile(shape, F32, name=name, tag=name, bufs=1)

        # persistent planes (plane-major SoA; plane stride = ncat or n16)
        scat = ptile([P, 3 * ncat], "scat")       # [3, ncat] hat log-sums
        dwal = ptile([P, 3 * n16], "dwal")        # [3, n16] subsampled dw
        gcat = ptile([P, 8 * ncat], "gcat")       # [8, ncat]: [g; -g]
        hcat = ptile([P, 4 * ncat], "hcat")       # [4, ncat] hat quats
        qcat = ptile([P, 4 * ncat], "qcat")       # [4, ncat] residual quats
        Mt = ptile([P, 4 * ncat * 4], "Mt")       # [4, ncat, 4] products
        sqd = ptile([P, 3 * n16], "sqd")
        a2t = ptile([P, n16], "a2t")
        a_t = ptile([P, n16], "a_t")
        diat = ptile([P, n16], "diat")
        sht = ptile([P, n16], "sht")
        k_t = ptile([P, n16], "k_t")
        sq2 = ptile([P, 3 * ncat], "sq2")
        s2n = ptile([P, ncat], "s2n")
        n2t = ptile([P, ncat], "n2t")
        t1t = ptile([P, ncat], "t1t")
        snct = ptile([P, ncat], "snct")
        w2t = ptile([P, ncat], "w2t")
        s2t = ptile([P, ncat], "s2t")
        lwt = ptile([P, ncat], "lwt")
        lvt = ptile([P, ncat], "lvt")
        dt0t = ptile([P, ncat], "dt0t")
        adtt = ptile([P, ncat], "adtt")
        tt_ = ptile([P, ncat], "tt_")
        ivt = ptile([P, ncat], "ivt")
        thpt = ptile([P, ncat], "thpt")
        selt = ptile([P, ncat], "selt")
        u1t = ptile([P, ncat], "u1t")
        tht = ptile([P, ncat], "tht")
        gft = ptile([P, ncat], "gft")
        abt = ptile([P, 3 * ncat], "abt")
        mmt = ptile([P, 3 * ncat], "mmt")
        mkc_t = ptile([P, ncat], "mkc")
        out_t = ptile([P, 24], "out_t")

        whf, dwf, gtf, mnf, sdf = (_flat(x) for x in (wh_d, dw_d, gt_d, mn_d, sd_d))

        def apv(tile, off, dims):
            # strided view of a tile: dims = [(stride, count), ...] (elements)
            base = tile[:]
            pstr, pcnt = base.ap[0]
            return AP(base.tensor, base.offset + off,
                      [[pstr, pcnt]] + [[s, n] for s, n in dims])

        def bcast3(plane_ap, w):
            # [P, w] -> [P, 3, w] stride-0 broadcast
            return plane_ap.rearrange("p (a g) -> p a g", a=1).broadcast_to(
                [P, 3, w])

        # ---- quaternion product: out rows from A[4 planes] x stack S[8 rows]
        def emit_qprod(rows, A_of, S_of, out_of, w):
            # A_of(i0, di, k) / S_of(s0, ds, k) -> [P, k, w] views;
            # out_of(c) -> [P, 4, w] destination (i innermost in Mt)
            for c, specs in rows.items():
                slot = 0
                for (i0, di, s0, ds, k) in specs:
                    v.tensor_tensor(
                        apv(Mt, c * 4 * w + slot, [(1, k), (4, w)]),
                        A_of(i0, di, k), S_of(s0, ds, k), Op.mult)
                    slot += k
            v.tensor_reduce(out_of(), apv(Mt, 0, [(4 * w, 4), (4, w), (1, 4)]),
                            axis=AX.X, op=Op.add)

        # ---------------- per-pass gyro tail ----------------
        KH = DT / 2

        def emit_tail(p_):
            w16 = pw16[p_]
            w32 = pw32[p_]
            w = w16 + w32
            a16 = pbase[p_]          # 16-level col base
            a32 = a16 + w16          # 32-level col base
            s_ = slice(a16, a16 + w)

            # s32 = pairwise sums of s16 (within pass)
            v.tensor_tensor(apv(scat, a32, [(ncat, 3), (1, w32)]),
                            apv(scat, a16, [(ncat, 3), (2, w32)]),
                            apv(scat, a16 + 1, [(ncat, 3), (2, w32)]), Op.add)

            # hat quats: 5th-order Taylor of exp((DT/2)|s|)
            act.activation(apv(sq2, a16, [(ncat, 3), (1, w)]),
                           apv(scat, a16, [(ncat, 3), (1, w)]), AF.Square)
            gp.tensor_tensor(s2n[:, s_], sq2[:, a16:a16 + w],
                             sq2[:, ncat + a16:ncat + a16 + w], Op.add)
            gp.tensor_tensor(s2n[:, s_], s2n[:, s_],
                             sq2[:, 2 * ncat + a16:2 * ncat + a16 + w], Op.add)
            act.activation(hcat[:, s_], s2n[:, s_], AF.Identity,
                           bias=1.0, scale=-KH * KH / 2)
            act.activation(snct[:, s_], s2n[:, s_], AF.Identity,
                           bias=KH, scale=-KH ** 3 / 6)
            if DBG_BC:
                v.tensor_tensor(apv(hcat, ncat + a16, [(ncat, 3), (1, w)]),
                                apv(scat, a16, [(ncat, 3), (1, w)]),
                                bcast3(snct[:, s_], w), Op.mult)
            if p_ == 0:
                v.tensor_tensor(
                    apv(hcat, a16, [(ncat, 4), (1, w)]),
                    apv(hcat, a16, [(ncat, 4), (1, w)]),
                    mkc_t[:, s_].rearrange("p (a g) -> p a g", a=1)
                        .broadcast_to([P, 4, w]), Op.mult)
            else:
                for ci in range(3):
                    v.tensor_tensor(hcat[:, (1 + ci) * ncat + a16:(1 + ci) * ncat + a16 + w],
                                    scat[:, ci * ncat + a16:ci * ncat + a16 + w],
                                    snct[:, s_], Op.mult)

            # g32 = pairwise products of g16 (even (x) odd)
            emit_qprod(
                ROWS_MUL,
                lambda i0, di, k: apv(gcat, i0 * ncat + a16, [(di * ncat, k), (2, w32)]),
                lambda s0, ds, k: apv(gcat, s0 * ncat + a16 + 1, [(ds * ncat, k), (2, w32)]),
                lambda: apv(gcat, a32, [(ncat, 4), (1, w32)]),
                w32)
            # stack rows 4-7 for the 32 cols
            act.mul(apv(gcat, 4 * ncat + a32, [(ncat, 4), (1, w32)]),
                    apv(gcat, a32, [(ncat, 4), (1, w32)]), -1.0)

            # residual = conj(hat) (x) gt over the full pass (16|32 fused)
            emit_qprod(
                ROWS_CONJ,
                lambda i0, di, k: apv(hcat, i0 * ncat + a16, [(di * ncat, k), (1, w)]),
                lambda s0, ds, k: apv(gcat, s0 * ncat + a16, [(ds * ncat, k), (1, w)]),
                lambda: apv(qcat, a16, [(ncat, 4), (1, w)]),
                w)

            # log: theta/2 = arctan(|v|/|w|) via exp/ln; 1/|v| = exp(-.5 ln s2)
            act.activation(w2t[:, s_], qcat[:, s_], AF.Square)
            v.tensor_scalar(s2t[:, s_], w2t[:, s_], -1.0, 1.0, Op.mult, Op.add)
            v.tensor_scalar(s2t[:, s_], s2t[:, s_], 1e-12, None, Op.max)
            v.tensor_scalar(w2t[:, s_], w2t[:, s_], 1e-12, None, Op.max)
            act.activation(lwt[:, s_], w2t[:, s_], AF.Ln)
            act.activation(lvt[:, s_], s2t[:, s_], AF.Ln)
            # |qv| off the critical chain (needs only the qprod result)
            act.activation(apv(mmt, a16, [(ncat, 3), (1, w)]),
                           apv(qcat, ncat + a16, [(ncat, 3), (1, w)]), AF.Abs)
            v.tensor_tensor(dt0t[:, s_], lvt[:, s_], lwt[:, s_], Op.subtract)
            act.activation(tt_[:, s_], dt0t[:, s_], AF.Exp, scale=0.5)
            act.activation(ivt[:, s_], lvt[:, s_], AF.Exp, scale=-0.5)
            act.activation(thpt[:, s_], tt_[:, s_], AF.Arctan)
            v.scalar_tensor_tensor(gft[:, s_], thpt[:, s_], 2.0 / H_,
                                   ivt[:, s_], Op.mult, Op.mult)

            # huber, batched over components: m*(2|t|-m), m = min(|t|, 1)
            v.tensor_tensor(apv(abt, a16, [(ncat, 3), (1, w)]),
                            apv(mmt, a16, [(ncat, 3), (1, w)]),
                            bcast3(gft[:, s_], w), Op.mult)
            v.tensor_scalar(apv(mmt, a16, [(ncat, 3), (1, w)]),
                            apv(abt, a16, [(ncat, 3), (1, w)]),
                            1.0, None, Op.min)
            v.scalar_tensor_tensor(apv(abt, a16, [(ncat, 3), (1, w)]),
                                   apv(abt, a16, [(ncat, 3), (1, w)]), 2.0,
                                   apv(mmt, a16, [(ncat, 3), (1, w)]),
                                   Op.mult, Op.subtract)
            v.tensor_tensor(apv(abt, a16, [(ncat, 3), (1, w)]),
                            apv(abt, a16, [(ncat, 3), (1, w)]),
                            apv(mmt, a16, [(ncat, 3), (1, w)]), Op.mult)
            v.tensor_reduce(out_t[:, 6 * p_:6 * p_ + 3],
                            apv(abt, a16, [(ncat, 3), (1, w16)]),
                            axis=AX.X, op=Op.add)
            v.tensor_reduce(out_t[:, 6 * p_ + 3:6 * p_ + 6],
                            apv(abt, a32, [(ncat, 3), (1, w32)]),
                            axis=AX.X, op=Op.add)

        # ---------------- streaming chunk loop ----------------
        with tc.tile_pool(name="io", bufs=2) as iop, \
             tc.tile_pool(name="wk", bufs=2) as wkp:
            CMAX = max(SIZES)
            off = 0
            for c, Cs in enumerate(SIZES):
                w3 = 3 * Cs
                csl = slice(off * 3, (off + Cs) * 3)
                sd_t = iop.tile([P, 3 * CMAX], F32, name="sd_t", tag="sd")
                nc.sync.dma_start(out=sd_t[:, :w3], in_=sdf[:, csl])
                gt_t = iop.tile([P, 3 * CMAX], F32, name="gt_t", tag="gt")
                nc.sync.dma_start(out=gt_t[:, :w3], in_=gtf[:, csl])
                wh_t = iop.tile([P, 3 * CMAX], F32, name="wh_t", tag="wh")
                nc.sync.dma_start(out=wh_t[:, :w3], in_=whf[:, csl])
                mn_t = iop.tile([P, 3 * CMAX], F32, name="mn_t", tag="mn")
                nc.sync.dma_start(out=mn_t[:, :w3], in_=mnf[:, csl])
                if c == 0:
                    nc.sync.dma_start(out=mkc_t[:], in_=mkc_d[:])
                if c == 1:
                    # strided 1-in-16 gather on the qAct queue; issued here so
                    # its completion-sem reuse lands on c3's loads (which are
                    # naturally later than the ~20us gather)
                    dwi = AP(dwf.tensor, dwf.offset,
                             [list(dwf.ap[0]), [48, n16], [1, 3]])
                    dwo = AP(dwal[:].tensor, dwal[:].offset,
                             [list(dwal[:].ap[0]), [3, n16], [1, 3]])
                    act.dma_start(out=dwo, in_=dwi)

                # gaussian NLL: subs on gpsimd, max/mul on vector, ln/exp on ACT
                Sc = wkp.tile([P, 3 * CMAX], F32, name="Sc", tag="Sc")
                act.activation(Sc[:, :w3], sd_t[:, :w3], AF.Relu, bias=-1e-3)
                lnS = wkp.tile([P, 3 * CMAX], F32, name="lnS", tag="lnS")
                act.activation(lnS[:, :w3], Sc[:, :w3], AF.Ln, bias=1e-3,
                               accum_out=out_t[:, 12 + c:13 + c])
                isd = Sc  # reuse (Sc dead after Ln)
                act.activation(isd[:, :w3], lnS[:, :w3], AF.Exp, scale=-1.0)
                d_t = wkp.tile([P, 3 * CMAX], F32, name="d_t", tag="d")
                gp.tensor_tensor(d_t[:, :w3], gt_t[:, :w3], wh_t[:, :w3],
                                 Op.subtract)
                gp.tensor_tensor(d_t[:, :w3], d_t[:, :w3], mn_t[:, :w3],
                                 Op.subtract)
                v.tensor_tensor(d_t[:, :w3], d_t[:, :w3], isd[:, :w3],
                                Op.mult)
                junk = lnS  # reuse (lnS dead after Exp)
                act.activation(junk[:, :w3], d_t[:, :w3], AF.Square,
                               accum_out=out_t[:, 17 + c:18 + c])

                # hat side: 16-group log-sum in ONE strided reduce ->
                # SoA scat slice (pass-major columns)
                g0, ng = off // 16, Cs // 16
                if DBG_RED:
                    v.tensor_reduce(
                        apv(scat, col16(g0), [(ncat, 3), (1, ng)]),
                        wh_t[:, :w3].rearrange("p (g k c) -> p c g k", k=16, c=3),
                        axis=AX.X, op=Op.add)
                else:
                    gp.memset(apv(scat, col16(g0), [(ncat, 3), (1, ng)]), 0.01)

                off += Cs

                if c == 2 and DBG_TAIL:
                    # ---- gt16 quats for ALL groups (dw complete); sits in
                    # the c3/c4 DMA shadow.  a = sqrt(a2) and 1/a both come
                    # from one Ln (natural_log_exp table, already loaded).
                    act.activation(sqd[:], dwal[:], AF.Square)
                    gp.tensor_tensor(a2t[:], apv(sqd, 0, [(3, n16)]),
                                     apv(sqd, 1, [(3, n16)]), Op.add)
                    gp.tensor_tensor(a2t[:], a2t[:], apv(sqd, 2, [(3, n16)]),
                                     Op.add)
                    v.tensor_scalar(a2t[:], a2t[:], 1e-12, None, Op.max)
                    act.activation(t1t[:, :n16], a2t[:], AF.Ln)
                    act.activation(diat[:], t1t[:, :n16], AF.Exp, scale=-0.5)
                    v.tensor_tensor(a_t[:], a2t[:], diat[:], Op.mult)
                    act.activation(sht[:], a_t[:], AF.Sin, bias=PI, scale=-0.5)
                    v.tensor_tensor(k_t[:], sht[:], diat[:], Op.mult)
                    for p_, (g0_, g1_) in enumerate(PASSES):
                        wg = g1_ - g0_
                        a16 = pbase[p_]
                        # qw = cos(a/2) = sin(pi/2 - a/2)
                        act.activation(gcat[:, a16:a16 + wg],
                                       a_t[:, g0_:g1_], AF.Sin,
                                       bias=PI / 2, scale=-0.5)
                        # qv = dw * sin(a/2)/a
                        v.tensor_tensor(
                            apv(gcat, ncat + a16, [(ncat, 3), (1, wg)]),
                            apv(dwal, 3 * g0_, [(1, 3), (3, wg)]),
                            bcast3(k_t[:, g0_:g1_], wg), Op.mult)
                        # stack rows 4-7 = -g16
                        act.mul(
                            apv(gcat, 4 * ncat + a16, [(ncat, 4), (1, wg)]),
                            apv(gcat, a16, [(ncat, 4), (1, wg)]), -1.0)
                    # pass A tail rides in the c3/c4 DMA shadow
                    emit_tail(0)

        if DBG_TAIL:
            emit_tail(1)
        nc.sync.dma_start(out=out_d[:], in_=out_t[:])

    return nc


def combine(parts, N, T):
    """parts: [..., 24] per-partition sums; see out_t layout in build()."""
    s = np.asarray(parts, dtype=np.float64).reshape(-1, 24).sum(axis=0)
    n16, n32 = T // 16, T // 32
    npass = len(PASSES)
    s16 = sum(s[6 * p_:6 * p_ + 3].sum() for p_ in range(npass))
    s32 = sum(s[6 * p_ + 3:6 * p_ + 6].sum() for p_ in range(npass))
    s_ln = s[12:12 + NCH].sum()
    s_u2 = s[17:17 + NCH].sum()
    gyro16 = W_ * H_ ** 2 * 0.5 * s16 / (N * (n16 - N0) * 3)
    gyro32 = (W_ * H_ ** 2 / 4) * 0.5 * s32 / (N * (n32 - N0) * 3)
    gnll = (2.0 * s_ln + s_u2) / (2.0 * N * T * 3)
    return np.array(gyro16 + gyro32 + gnll, dtype=np.float32)


_NC_CACHE = {}


def last_exec_time_ns():
    res = _NC_CACHE.get("last_res")
    if res is None:
        return None
    return res.exec_time_ns or res.mean_exec_time_ns


def make_maskc(n_seq, T):
    sp = P // n_seq
    L = T // sp
    n16, n32 = L // 16, L // 32
    mk = np.ones((P, n16 + n32), dtype=np.float32)
    base = 0
    for (g0, g1) in PASSES:
        w16 = g1 - g0
        w32 = w16 // 2
        for j in range(w16):
            if g0 + j < N0:
                mk[::sp, base + j] = 0.0
        for j in range(w32):
            if g0 // 2 + j < N0:
                mk[::sp, base + w16 + j] = 0.0
        base += w16 + w32
    return mk


def _register_ntff_shim():
    import sys, types
    try:
        import antenv.axon_hooks  # noqa: F401
        return
    except ImportError:
        pass
    from trn_agent_boot.trn_boot import _ntff_profile_via_ctypes
    hook = _ntff_profile_via_ctypes('/opt/axon/libaxon_pjrt.so')
    mod = types.ModuleType("antenv.axon_hooks")
    mod.get_axon_ntff_profile_hook = lambda: hook
    import antenv
    antenv.axon_hooks = mod
    sys.modules["antenv.axon_hooks"] = mod


def kernel(w_hat, dw_16, w_gt, w_mean, w_std):
    import os
    from concourse.bass_utils import run_bass_kernel_spmd
    if os.environ.get("KERNEL_PROFILE"):
        _register_ntff_shim()

    if "nc" not in _NC_CACHE:
        nc_ = build(N_FULL // N_CORES, T_FULL)
        _split_multi_waits(nc_)
        _NC_CACHE["nc"] = nc_
    nc = _NC_CACHE["nc"]

    mkc = make_maskc(N_FULL // N_CORES, T_FULL)
    spc = N_FULL // N_CORES
    ins = dict(w_hat=w_hat, dw_16=dw_16, w_gt=w_gt, w_mean=w_mean, w_std=w_std)
    in_maps = []
    for c in range(N_CORES):
        m = {k: np.ascontiguousarray(
            np.asarray(a, dtype=np.float32)[c * spc:(c + 1) * spc])
            for k, a in ins.items()}
        m["maskc"] = mkc
        in_maps.append(m)
    res = run_bass_kernel_spmd(nc, in_maps, list(range(N_CORES)),
                               trace=bool(os.environ.get("KERNEL_PROFILE")))
    _NC_CACHE["last_res"] = res
    parts = np.stack([r["out"] for r in res.results])
    return combine(parts, N_FULL, T_FULL)

